# revision 3
# baseline (speedup 1.0000x reference)
"""CapsuleMaxPooling Trainium2 kernel.

Problem: inp [B=32, C=32, H=64, W=64, D=8] f32, kernel_size k=2.
For each 2x2 spatial window pick the capsule vector (length D=8) with the
largest squared L2 norm (first-max tie-break) -> out [B, C, 32, 32, 8].

Strategy (fully data-parallel, shard B across 8 cores; per core the shard is
viewed as rows r=(b, c, hk) of 1024 contiguous floats = (dh, wk, dw, d),
i.e. both H-rows of all windows in that row; 32 row-tiles of 128 partitions).

The 20 MiB/core of HBM traffic bounds the kernel at ~59us, so the compute
is spread over engines to keep each below that roofline:
  - ACT: sq = x^2 (Square activation) + base copy of candidate D into the
    output tile.
  - DVE: tree level-1 add of sq halves (8->4) -- tensor_tensor streams both
    read ports so this beats a grouped tensor_reduce 2x; the tournament
    (max / is_ge masks); 3x copy_predicated. copy_predicated wants an
    integer mask: we hand it an int32 bitcast view of the f32 0.0/1.0 mask
    (1.0f = 0x3F800000 != 0) broadcast over d via a stride-0 inner dim.
  - GPSIMD (Pool): tree levels 2+3 (4->2->1); only `add` is supported
    there. Level 3 writes norms transposed to [pos, wk] so the tournament
    reads contiguously. Predication ORDER (D base, then C, then B, then A
    last) yields exact first-argmax.
  - HWDGE (nc.sync) DMAs, partition-major: each partition reads one
    contiguous 4KB*tb chunk and writes one contiguous 1KB*tb chunk.
  - Groups are software-pipelined by one: group g's masks/copies issue
    after group g+1's loads, so the DVE never stalls waiting for masks.
"""

import numpy as np

try:
    import concourse.bass as bass
except ImportError:  # pragma: no cover
    import sys

    sys.path.insert(0, "/opt/trn_rl_repo")
    import concourse.bass as bass

from concourse import bacc, mybir
from concourse.bass_utils import run_bass_kernel_spmd
from concourse.tile import TileContext

P = 128
N_CORES = 8
ROW_W = 1024  # (dh=2) * (wk=32) * (dw=2) * (d=8)
OUT_W = 256  # (wk=32) * (d=8)
DEFAULT_SCHED = (1, 1, 2, 2, 4, 4, 4, 4, 4, 4, 2)


def _bcs(w, n):
    """View mask tile slice w [P, tb, 32] as int32 [P, tb, 32, n] via a
    bitcast + stride-0 inner dim."""
    a = w.bitcast(mybir.dt.int32)
    return bass.AP(tensor=a.tensor, offset=a.offset, ap=[*a.ap, [0, n]])


def build_nc(R=4096, sched=DEFAULT_SCHED, GM=2):
    """Build the per-core Bass program. R = rows (b,c,hk) per core."""
    f32 = mybir.dt.float32
    add = mybir.AluOpType.add
    nc = bacc.Bacc(None, target_bir_lowering=False)
    x = nc.dram_tensor("x", [R, ROW_W], f32, kind="ExternalInput")
    y = nc.dram_tensor("y", [R, OUT_W], f32, kind="ExternalOutput")
    assert sum(sched) * P == R
    groups = [list(sched[i : i + GM]) for i in range(0, len(sched), GM)]

    with TileContext(nc) as tc:
        with (
            tc.tile_pool(name="xp", bufs=6) as xp,
            tc.tile_pool(name="sqp", bufs=2) as sqp,
            tc.tile_pool(name="s4p", bufs=2) as s4p,
            tc.tile_pool(name="s2p", bufs=2) as s2p,
            tc.tile_pool(name="normp", bufs=2) as normp,
            tc.tile_pool(name="maskp", bufs=2) as maskp,
            tc.tile_pool(name="outp", bufs=6) as outp,
        ):

            def load_group(grp, tile0):
                """DMA-in + squares + tree reduce + base copy for one group.
                Returns state needed by select_group."""
                gtb = sum(grp)
                nt = normp.tile([P, gtb, 4, 32], f32, tag="nt")
                xts, ots, qoff = [], [], [0]
                t = tile0
                for tb in grp:
                    r0 = t * P
                    xt = xp.tile([P, tb, ROW_W], f32, tag="xt")
                    xts.append(xt)
                    nc.sync.dma_start(
                        out=xt,
                        in_=x[r0 : r0 + tb * P, :].rearrange(
                            "(p j) c -> p j c", p=P
                        ),
                    )
                    sq = sqp.tile([P, tb, ROW_W], f32, tag="sq")
                    nc.scalar.square(sq, xt)
                    # level 1 on DVE: d 8 -> 4
                    sqv = sq.rearrange("p j (g d) -> p j g d", d=8)
                    s4 = s4p.tile([P, tb, 128, 4], f32, tag="s4")
                    nc.vector.tensor_tensor(
                        s4, sqv[:, :, :, 0:4], sqv[:, :, :, 4:8], op=add
                    )
                    # levels 2+3 on gpsimd: 4 -> 2 -> 1 (transposed out)
                    s2 = s2p.tile([P, tb, 128, 2], f32, tag="s2")
                    nc.gpsimd.tensor_tensor(
                        s2, s4[:, :, :, 0:2], s4[:, :, :, 2:4], op=add
                    )
                    s2v = s2.rearrange(
                        "p j (dh wk dw) e -> p j dh wk dw e", dh=2, wk=32
                    )
                    ntv = nt[:, qoff[-1] : qoff[-1] + tb].rearrange(
                        "p j (dh dw) wk -> p j dh wk dw", dh=2
                    )
                    nc.gpsimd.tensor_tensor(
                        ntv, s2v[:, :, :, :, :, 0], s2v[:, :, :, :, :, 1],
                        op=add,
                    )
                    # base candidate D into the output tile (ACT)
                    ot = outp.tile([P, tb, 32, 8], f32, tag="ot")
                    ots.append(ot)
                    xr = xt.rearrange(
                        "p j (dh wk dw d) -> p j dh wk dw d", dh=2, dw=2, d=8
                    )
                    nc.scalar.copy(ot, xr[:, :, 1, :, 1, :])
                    qoff.append(qoff[-1] + tb)
                    t += tb
                return dict(
                    grp=grp, gtb=gtb, nt=nt, xts=xts, ots=ots, qoff=qoff,
                    tile0=tile0,
                )

            def select_group(st):
                """Tournament + predicated copies + DMA-out for one group."""
                grp, gtb, nt = st["grp"], st["gtb"], st["nt"]
                nA = nt[:, :, 0, :]
                nB = nt[:, :, 1, :]
                nC = nt[:, :, 2, :]
                nD = nt[:, :, 3, :]
                h1 = maskp.tile([P, gtb, 32], f32, tag="h1")
                nc.vector.tensor_tensor(h1, nA, nB, op=mybir.AluOpType.max)
                h2 = maskp.tile([P, gtb, 32], f32, tag="h2")
                nc.vector.tensor_tensor(h2, nC, nD, op=mybir.AluOpType.max)
                M = maskp.tile([P, gtb, 32], f32, tag="M")
                nc.vector.tensor_tensor(M, h1, h2, op=mybir.AluOpType.max)
                wA = maskp.tile([P, gtb, 32], f32, tag="wA")
                nc.vector.tensor_tensor(wA, nA, M, op=mybir.AluOpType.is_ge)
                wB = maskp.tile([P, gtb, 32], f32, tag="wB")
                nc.vector.tensor_tensor(wB, nB, M, op=mybir.AluOpType.is_ge)
                wC = maskp.tile([P, gtb, 32], f32, tag="wC")
                nc.vector.tensor_tensor(wC, nC, M, op=mybir.AluOpType.is_ge)

                t = st["tile0"]
                for qi, tb in enumerate(grp):
                    r0 = t * P
                    xt = st["xts"][qi]
                    ot = st["ots"][qi]
                    xr = xt.rearrange(
                        "p j (dh wk dw d) -> p j dh wk dw d", dh=2, dw=2, d=8
                    )
                    Av = xr[:, :, 0, :, 0, :]
                    Bv = xr[:, :, 0, :, 1, :]
                    Cv = xr[:, :, 1, :, 0, :]
                    q0, q1 = st["qoff"][qi], st["qoff"][qi] + tb
                    nc.vector.copy_predicated(ot, _bcs(wC[:, q0:q1], 8), Cv)
                    nc.vector.copy_predicated(ot, _bcs(wB[:, q0:q1], 8), Bv)
                    nc.vector.copy_predicated(ot, _bcs(wA[:, q0:q1], 8), Av)

                    nc.sync.dma_start(
                        out=y[r0 : r0 + tb * P, :].rearrange(
                            "(p j) c -> p j c", p=P
                        ),
                        in_=ot.rearrange("p j w d -> p j (w d)"),
                    )
                    t += tb

            pend = None
            tile0 = 0
            for grp in groups:
                st = load_group(grp, tile0)
                tile0 += sum(grp)
                if pend is not None:
                    select_group(pend)
                pend = st
            select_group(pend)
    nc.compile()
    return nc


_NC_CACHE = {}


def _get_nc(R):
    if R not in _NC_CACHE:
        _NC_CACHE[R] = build_nc(R)
    return _NC_CACHE[R]


def kernel(inp, kernel_size):
    inp = np.asarray(inp)
    k = int(np.asarray(kernel_size))
    assert k == 2, f"kernel hardcoded for kernel_size=2, got {k}"
    B, C, H, W, D = inp.shape
    assert (B, C, H, W, D) == (32, 32, 64, 64, 8), inp.shape
    Hk, Wk = H // k, W // k

    bs = B // N_CORES  # 4 batches per core
    R = bs * C * Hk  # 4096 rows per core
    nc = _get_nc(R)

    in_maps = []
    for c in range(N_CORES):
        shard = np.ascontiguousarray(inp[c * bs : (c + 1) * bs]).reshape(R, ROW_W)
        in_maps.append({"x": shard})

    res = run_bass_kernel_spmd(nc, in_maps, list(range(N_CORES)))
    out = np.concatenate(
        [r["y"].reshape(bs, C, Hk, Wk, D) for r in res.results], axis=0
    )
    return out
